# revision 3
# baseline (speedup 1.0000x reference)
"""Bahdanau attention kernel for Trainium2, 8-core data-parallel.

Problem (per full batch): B=4, T=128, S=512, H=512, fp32.
  q_proj = query @ W_s.T ; k_proj = enc @ W_h.T
  score[t,s] = sum_h v[h] * tanh(q_proj[t,h] + k_proj[s,h])  (+ length mask)
  attn = softmax_s(score); context = attn @ enc
  out = LN(tanh([context, query] @ W_out.T + b_out)) * gamma + beta

Sharding: 8 cores = (batch b = core//2) x (T-half = core%2), each core owns a
(64, 512) slice of the output. Weights replicated. All transposed layouts are
prepared on the host so the device does no weight/activation transposes except
a single 4-tile PE transpose of the attention matrix.

Device algorithm per core (o = projection dim, chunked 4 x 128):
  phase 1: k_projT (o,s) and q_projT (o,t) via PE matmuls (fp32).
  phase 2: for each o-chunk c, group of 8 t's:
             DVE tensor_scalar_add: arg[:, j*512:(j+1)*512] = k_projT[c] + q_projT[c][:,t]
             ACT tanh over (128, 4096) fp32 -> bf16
             PE: score_psum(64,512) += onehot_v[t,c].T @ tanh_tile   (bf16)
           The one-hot lhsT trick lands score[t, :] on PSUM partition t while
           the matmul still streams N=512 columns per instruction.
           The length mask is folded in as a K=1 matmul ones(1,64).T @ mask(1,512).
  phase 3: row softmax on PSUM (64,512): reduce_max, ACT exp (bias=-max,
           accum_out=rowsum), DVE reciprocal, tensor_scalar_mul.
  phase 4: PE-transpose attn -> attnT; contextT[h,t] = enc[s,h].T-chunks @ attnT.
  phase 5: out_pre = [contextT; queryT].T @ W_outT (+ b_out via K=1 matmul), tanh.
  phase 6: LayerNorm via bn_stats/bn_aggr, sqrt(var+eps) on ACT, DVE reciprocal,
           tensor_scalar(sub,mult), then * gamma + beta (replicated tiles).
"""

import numpy as np
import ml_dtypes

import concourse.bass as bass
import concourse.tile as tile
from concourse import bacc, mybir
from concourse.bass import ts
from concourse.bass_utils import run_bass_kernel_spmd
from concourse.masks import make_identity

B, T, S, H = 4, 128, 512, 512
NCORES = 8
TSH = T // 2          # 64 t-rows per core
H2 = 2 * H
LN_EPS = 1e-5
MASK_VAL = -1e9

F32 = mybir.dt.float32
BF16 = mybir.dt.bfloat16
AF = mybir.ActivationFunctionType
ALU = mybir.AluOpType

NCHUNK = H // 128     # 4 o-chunks / h-chunks / s-chunks
TGRP = 8              # t's per ACT group
NGRP = TSH // TGRP    # 8 groups


_LAST_NC = None


def build_program() -> bacc.Bacc:
    nc = bacc.Bacc("TRN2", target_bir_lowering=False, debug=False)

    encT_d = nc.dram_tensor("encT", [H, S], F32, kind="ExternalInput")
    enc_d = nc.dram_tensor("enc", [S, H], F32, kind="ExternalInput")
    qT_d = nc.dram_tensor("qT", [H, TSH], F32, kind="ExternalInput")
    whT_d = nc.dram_tensor("whT", [H, H], F32, kind="ExternalInput")
    wsT_d = nc.dram_tensor("wsT", [H, H], F32, kind="ExternalInput")
    woT_d = nc.dram_tensor("woT", [H2, H], F32, kind="ExternalInput")
    vc_d = nc.dram_tensor("vc", [128, NCHUNK], F32, kind="ExternalInput")
    mask_d = nc.dram_tensor("mask", [1, S], BF16, kind="ExternalInput")
    bout_d = nc.dram_tensor("bout", [1, H], F32, kind="ExternalInput")
    gam_d = nc.dram_tensor("gam", [TSH, H], F32, kind="ExternalInput")
    bet_d = nc.dram_tensor("bet", [TSH, H], F32, kind="ExternalInput")
    out_d = nc.dram_tensor("out", [TSH, H], F32, kind="ExternalOutput")

    with tile.TileContext(nc) as tc:
        with (
            tc.tile_pool(name="const", bufs=1) as const,
            tc.tile_pool(name="work", bufs=1) as work,
            tc.tile_pool(name="argp", bufs=3) as argp,
            tc.tile_pool(name="thp", bufs=3) as thp,
            tc.tile_pool(name="ps", bufs=4, space="PSUM") as psp,
            tc.tile_pool(name="pscore", bufs=1, space="PSUM") as pscore,
        ):
            # ---- constant loads -------------------------------------------
            def load(dram, shape, dtype, tag):
                t_ = const.tile(shape, dtype, tag=tag)
                nc.sync.dma_start(out=t_[:], in_=dram)
                return t_

            whT = [load(whT_d[ts(c, 128), :], [128, H], F32, f"whT{c}") for c in range(NCHUNK)]
            wsT = [load(wsT_d[ts(c, 128), :], [128, H], F32, f"wsT{c}") for c in range(NCHUNK)]
            encT = [load(encT_d[ts(c, 128), :], [128, S], F32, f"encT{c}") for c in range(NCHUNK)]
            enc = [load(enc_d[ts(c, 128), :], [128, H], F32, f"enc{c}") for c in range(NCHUNK)]
            qT = [load(qT_d[ts(c, 128), :], [128, TSH], F32, f"qT{c}") for c in range(NCHUNK)]
            woT = [load(woT_d[ts(c, 128), :], [128, H], F32, f"woT{c}") for c in range(2 * NCHUNK)]
            vc = load(vc_d[:, :], [128, NCHUNK], F32, "vc")
            maskv = load(mask_d[:, :], [1, S], BF16, "maskv")
            bout = load(bout_d[:, :], [1, H], F32, "bout")
            gam = load(gam_d[:, :], [TSH, H], F32, "gam")
            bet = load(bet_d[:, :], [TSH, H], F32, "bet")

            ident = const.tile([128, 128], F32, tag="ident")
            make_identity(nc, ident)
            ones_bf = const.tile([1, TSH], BF16, tag="ones_bf")
            nc.vector.memset(ones_bf, 1.0)
            ones_f = const.tile([1, TSH], F32, tag="ones_f")
            nc.vector.memset(ones_f, 1.0)
            ones64_bf = const.tile([128, TSH], BF16, tag="ones64_bf")
            nc.vector.memset(ones64_bf, 1.0)
            eps_t = const.tile([TSH, 1], F32, tag="eps")
            nc.vector.memset(eps_t, LN_EPS)

            # one-hot v tiles: oh[c][:, t*64 + m] = v[c*128+p] iff m == t
            oh = []
            for c in range(NCHUNK):
                oc = const.tile([128, TSH * TSH], BF16, tag=f"oh{c}")
                nc.vector.memset(oc, 0.0)
                diag = oc[:, 0 : TSH * TSH : TSH + 1]  # stride 65, count 64
                nc.vector.tensor_scalar_mul(out=diag, in0=ones64_bf, scalar1=vc[:, c : c + 1])
                oh.append(oc)

            # ---- phase 1: projections -------------------------------------
            kT = []
            qpT = []
            for c in range(NCHUNK):
                kp = psp.tile([128, S], F32, tag="ps")
                for hc in range(NCHUNK):
                    nc.tensor.matmul(
                        kp[:], whT[hc][:, ts(c, 128)], encT[hc][:],
                        start=(hc == 0), stop=(hc == NCHUNK - 1),
                    )
                kc_sb = work.tile([128, S], F32, tag=f"kT{c}")
                nc.vector.tensor_copy(out=kc_sb[:], in_=kp[:])
                kT.append(kc_sb)

                qp = psp.tile([128, TSH], F32, tag="ps")
                for hc in range(NCHUNK):
                    nc.tensor.matmul(
                        qp[:], wsT[hc][:, ts(c, 128)], qT[hc][:],
                        start=(hc == 0), stop=(hc == NCHUNK - 1),
                    )
                qc_sb = work.tile([128, TSH], F32, tag=f"qpT{c}")
                nc.vector.tensor_copy(out=qc_sb[:], in_=qp[:])
                qpT.append(qc_sb)

            # ---- phase 2: masked scores -----------------------------------
            score = pscore.tile([TSH, S], F32, tag="score")
            nc.tensor.matmul(score[:], ones_bf[:], maskv[:], start=True, stop=False)
            for c in range(NCHUNK):
                for g in range(NGRP):
                    arg = argp.tile([128, TGRP * S], F32, tag="arg")
                    for j in range(TGRP):
                        t_ = g * TGRP + j
                        nc.vector.tensor_scalar_add(
                            out=arg[:, ts(j, S)], in0=kT[c][:],
                            scalar1=qpT[c][:, t_ : t_ + 1],
                        )
                    th = thp.tile([128, TGRP * S], BF16, tag="th")
                    nc.scalar.activation(out=th[:], in_=arg[:], func=AF.Tanh)
                    for j in range(TGRP):
                        t_ = g * TGRP + j
                        last = (c == NCHUNK - 1) and (g == NGRP - 1) and (j == TGRP - 1)
                        nc.tensor.matmul(
                            score[:], oh[c][:, ts(t_, TSH)], th[:, ts(j, S)],
                            start=False, stop=last,
                        )

            # ---- phase 3: softmax over s ----------------------------------
            mx = work.tile([TSH, 1], F32, tag="mx")
            nc.vector.reduce_max(out=mx[:], in_=score[:], axis=mybir.AxisListType.X)
            nmx = work.tile([TSH, 1], F32, tag="nmx")
            nc.vector.tensor_scalar_mul(out=nmx[:], in0=mx[:], scalar1=-1.0)
            attn = work.tile([TSH, S], F32, tag="attn")
            sume = work.tile([TSH, 1], F32, tag="sume")
            nc.scalar.activation(
                out=attn[:], in_=score[:], func=AF.Exp, bias=nmx[:], accum_out=sume[:]
            )
            rec = work.tile([TSH, 1], F32, tag="rec")
            nc.vector.reciprocal(out=rec[:], in_=sume[:])
            nc.vector.tensor_scalar_mul(out=attn[:], in0=attn[:], scalar1=rec[:])

            # ---- phase 4: contextT = encT-chunks @ attnT ------------------
            attnT = []
            for sc in range(NCHUNK):
                ap_ps = psp.tile([128, TSH], F32, tag="ps")
                nc.tensor.transpose(ap_ps[:], attn[:, ts(sc, 128)], ident[:TSH, :TSH])
                at_sb = work.tile([128, TSH], F32, tag=f"attnT{sc}")
                nc.vector.tensor_copy(out=at_sb[:], in_=ap_ps[:])
                attnT.append(at_sb)
            ctxT = []
            for hc in range(NCHUNK):
                cp = psp.tile([128, TSH], F32, tag="ps")
                for sc in range(NCHUNK):
                    nc.tensor.matmul(
                        cp[:], enc[sc][:, ts(hc, 128)], attnT[sc][:],
                        start=(sc == 0), stop=(sc == NCHUNK - 1),
                    )
                ct_sb = work.tile([128, TSH], F32, tag=f"ctxT{hc}")
                nc.vector.tensor_copy(out=ct_sb[:], in_=cp[:])
                ctxT.append(ct_sb)

            # ---- phase 5: output projection + tanh ------------------------
            op = psp.tile([TSH, H], F32, tag="ps")
            for kc in range(2 * NCHUNK):
                lhsT = ctxT[kc] if kc < NCHUNK else qT[kc - NCHUNK]
                nc.tensor.matmul(op[:], lhsT[:], woT[kc][:], start=(kc == 0), stop=False)
            nc.tensor.matmul(op[:], ones_f[:], bout[:], start=False, stop=True)
            outt = work.tile([TSH, H], F32, tag="outt")
            nc.scalar.activation(out=outt[:], in_=op[:], func=AF.Tanh)

            # ---- phase 6: LayerNorm ---------------------------------------
            stats = work.tile([TSH, 6], F32, tag="stats")
            nc.vector.bn_stats(out=stats[:], in_=outt[:])
            mv = work.tile([TSH, 2], F32, tag="mv")
            nc.vector.bn_aggr(out=mv[:], in_=stats[:])
            std = work.tile([TSH, 1], F32, tag="std")
            nc.scalar.activation(out=std[:], in_=mv[:, 1:2], func=AF.Sqrt, bias=eps_t[:])
            rstd = work.tile([TSH, 1], F32, tag="rstd")
            nc.vector.reciprocal(out=rstd[:], in_=std[:])
            y = work.tile([TSH, H], F32, tag="y")
            nc.vector.tensor_scalar(
                out=y[:], in0=outt[:], scalar1=mv[:, 0:1], scalar2=rstd[:],
                op0=ALU.subtract, op1=ALU.mult,
            )
            nc.vector.tensor_mul(out=y[:], in0=y[:], in1=gam[:])
            nc.vector.tensor_add(out=y[:], in0=y[:], in1=bet[:])
            nc.sync.dma_start(out=out_d[:], in_=y[:])

    nc.compile()
    global _LAST_NC
    _LAST_NC = nc
    return nc


def shard_inputs(inputs: dict) -> list[dict]:
    query = np.ascontiguousarray(inputs["query"], dtype=np.float32)
    enc = np.ascontiguousarray(inputs["encoder_outputs"], dtype=np.float32)
    src_lengths = np.asarray(inputs["src_lengths"])
    W_h = np.ascontiguousarray(inputs["W_h"], dtype=np.float32)
    W_s = np.ascontiguousarray(inputs["W_s"], dtype=np.float32)
    v = np.ascontiguousarray(inputs["v"], dtype=np.float32)
    W_out = np.ascontiguousarray(inputs["W_out"], dtype=np.float32)
    b_out = np.ascontiguousarray(inputs["b_out"], dtype=np.float32)
    gamma = np.ascontiguousarray(inputs["gamma"], dtype=np.float32)
    beta = np.ascontiguousarray(inputs["beta"], dtype=np.float32)

    whT = np.ascontiguousarray(W_h.T)
    wsT = np.ascontiguousarray(W_s.T)
    woT = np.ascontiguousarray(W_out.T)
    vc = np.ascontiguousarray(v.reshape(NCHUNK, 128).T)
    bout = b_out.reshape(1, H)
    gam = np.ascontiguousarray(np.broadcast_to(gamma, (TSH, H)))
    bet = np.ascontiguousarray(np.broadcast_to(beta, (TSH, H)))

    in_maps = []
    for core in range(NCORES):
        b, half = divmod(core, 2)
        mask = np.where(
            np.arange(S) >= int(src_lengths[b]), np.float32(MASK_VAL), np.float32(0.0)
        ).astype(ml_dtypes.bfloat16).reshape(1, S)
        in_maps.append({
            "encT": np.ascontiguousarray(enc[b].T),
            "enc": enc[b],
            "qT": np.ascontiguousarray(query[b, half * TSH : (half + 1) * TSH, :].T),
            "whT": whT,
            "wsT": wsT,
            "woT": woT,
            "vc": vc,
            "mask": mask,
            "bout": bout,
            "gam": gam,
            "bet": bet,
        })
    return in_maps


def unshard(outs: list[np.ndarray]) -> np.ndarray:
    full = np.zeros((B, T, H), dtype=np.float32)
    for core in range(NCORES):
        b, half = divmod(core, 2)
        full[b, half * TSH : (half + 1) * TSH, :] = outs[core]
    return full


def kernel(**inputs) -> np.ndarray:
    nc = build_program()
    in_maps = shard_inputs(inputs)
    res = run_bass_kernel_spmd(nc, in_maps, list(range(NCORES)))
    return unshard([r["out"] for r in res.results])
